# revision 16
# baseline (speedup 1.0000x reference)
"""MoE FFN (8 experts, top-2) Trainium2 Bass kernel, expert-parallel over 8 cores.

Strategy (hardcoded for x[4,2048,1024] f32, E=8, H=2048, K=2):
  - Each core owns one expert's weights (w1[p], w2[p] in fp16).
  - Every core computes the fp32 gate (softmax + top-2 threshold) for all
    T=8192 tokens on-device, derives its own expert's routing mask/weights,
    stream-compacts the routed token ids via a triangular-matmul prefix sum,
    and scatters (token_id, gate_weight) pairs into a packed DRAM table with
    indirect DMA.
  - FFN: indirect-DMA row gather of routed tokens (fp16), PE transpose to
    [emb, tok] tiles, two fp16 matmul layers (relu fused into PSUM eviction),
    per-token gate-weight scaling on-chip, packed output written back.
  - Host combine: scatter-add each core's packed [tok_slot, 1024] output into
    the full [8192, 1024] result (token slots are unique per core).
"""

import numpy as np
from contextlib import ExitStack

import concourse.bass as bass
import concourse.bacc as bacc
import concourse.mybir as mybir
import concourse.tile as tile
from concourse.bass import IndirectOffsetOnAxis
from concourse.bass_utils import run_bass_kernel_spmd

F32 = mybir.dt.float32
F16 = mybir.dt.float16
I32 = mybir.dt.int32

T = 8192          # tokens
N = 1024          # embed
H = 2048          # hidden per expert
E = 8             # experts = cores
P = 128
NCH = T // P      # 64 routing chunks
CAP = 2304        # per-expert token capacity (mean 2048, +6.5 sigma)
CCH = CAP // P    # 18 packed chunks
# token chunks for matmul moving dim
TCH = [(o, min(512, CAP - o)) for o in range(0, CAP, 512)]


def build_program():
    nc = bacc.Bacc("TRN2")

    # ---------------- I/O ----------------
    xT = nc.dram_tensor("xT", [N, T], F32, kind="ExternalInput")
    xr16 = nc.dram_tensor("xr16", [T, N], F16, kind="ExternalInput")
    gwr = nc.dram_tensor("gwr", [P, 8, E], F32, kind="ExternalInput")
    gbr = nc.dram_tensor("gbr", [P, E], F32, kind="ExternalInput")
    w1r = nc.dram_tensor("w1r", [H // P, P, N // P, P], F16, kind="ExternalInput")
    w2r = nc.dram_tensor("w2r", [N // P, P, H // P, P], F16, kind="ExternalInput")
    b1r = nc.dram_tensor("b1r", [P, H // P], F32, kind="ExternalInput")
    b2r = nc.dram_tensor("b2r", [P, N // P], F32, kind="ExternalInput")
    esel = nc.dram_tensor("esel", [P, NCH * E], F32, kind="ExternalInput")
    LTc = nc.dram_tensor("LT", [P, P], F32, kind="ExternalInput")
    TOKID = nc.dram_tensor("TOKID", [P, NCH], F32, kind="ExternalInput")
    EYE16 = nc.dram_tensor("EYE16", [P, P], F16, kind="ExternalInput")
    EYE32 = nc.dram_tensor("EYE32", [P, P], F32, kind="ExternalInput")

    Y = nc.dram_tensor("Y", [N, CAP], F16, kind="ExternalOutput")
    IW = nc.dram_tensor("IW", [CAP + 1, 2], F32, kind="ExternalOutput")
    wscr = nc.dram_tensor("wscr", [CAP], F32)

    with tile.TileContext(nc) as tc, ExitStack() as ctx:
        cpool = ctx.enter_context(tc.tile_pool(name="const", bufs=1))
        gxpool = ctx.enter_context(tc.tile_pool(name="gatex", bufs=4))
        rpool = ctx.enter_context(tc.tile_pool(name="route", bufs=1))
        rtmp = ctx.enter_context(tc.tile_pool(name="rtmp", bufs=3))
        xgpool = ctx.enter_context(tc.tile_pool(name="xg", bufs=3))
        xgTpool = ctx.enter_context(tc.tile_pool(name="xgT", bufs=1))
        hTpool = ctx.enter_context(tc.tile_pool(name="hT", bufs=1))
        w1pool = ctx.enter_context(tc.tile_pool(name="w1s", bufs=2))
        w2pool = ctx.enter_context(tc.tile_pool(name="w2s", bufs=2))
        wbcpool = ctx.enter_context(tc.tile_pool(name="wbc", bufs=1))
        iwpool = ctx.enter_context(tc.tile_pool(name="iwc", bufs=3))
        evpool = ctx.enter_context(tc.tile_pool(name="ev", bufs=2))
        pp = ctx.enter_context(tc.tile_pool(name="ps", bufs=5, space="PSUM"))
        ptr = ctx.enter_context(tc.tile_pool(name="ptr", bufs=2, space="PSUM"))

        # ---------------- constants to SBUF ----------------
        gwr_sb = cpool.tile([P, 8, E], F32, tag="gwr")
        nc.sync.dma_start(gwr_sb[:], gwr[:])
        gbr_sb = cpool.tile([P, E], F32, tag="gbr")
        nc.sync.dma_start(gbr_sb[:], gbr[:])
        b1_sb = cpool.tile([P, H // P], F32, tag="b1")
        nc.sync.dma_start(b1_sb[:], b1r[:])
        b2_sb = cpool.tile([P, N // P], F32, tag="b2")
        nc.sync.dma_start(b2_sb[:], b2r[:])
        esel_sb = cpool.tile([P, NCH, E], F32, tag="esel")
        nc.sync.dma_start(esel_sb[:], esel[:].rearrange("p (c e) -> p c e", e=E))
        LT_sb = cpool.tile([P, P], F32, tag="LT")
        nc.sync.dma_start(LT_sb[:], LTc[:])
        tok_sb = cpool.tile([P, NCH], F32, tag="tokid")
        nc.sync.dma_start(tok_sb[:], TOKID[:])
        eye16_sb = cpool.tile([P, P], F16, tag="eye16")
        nc.sync.dma_start(eye16_sb[:], EYE16[:])
        eye32_sb = cpool.tile([P, P], F32, tag="eye32")
        nc.sync.dma_start(eye32_sb[:], EYE32[:])

        # ---------------- phase 1: gate logits (fp32) ----------------
        # psum_gate[tok128, chunk, expert]; per token chunk, accumulate the 8
        # emb k-chunks back-to-back (sim rejects interleaved psum groups).
        pgate = pp.tile([P, NCH, E], F32, tag="bank")
        for t in range(NCH):
            xblk = gxpool.tile([P, 8, P], F32, tag="slab")
            nc.sync.dma_start(
                xblk[:], xT[:, t * P:(t + 1) * P].rearrange("(a p) j -> p a j", p=P))
            for a in range(8):
                nc.tensor.matmul(
                    pgate[:, t, :],
                    lhsT=xblk[:, a, :],
                    rhs=gwr_sb[:, a, :],
                    start=(a == 0),
                    stop=(a == 7),
                )

        # ---------------- phase 2: softmax + top2 + own weight ----------------
        logits = rpool.tile([P, NCH, E], F32, tag="logits")
        nc.vector.tensor_tensor(
            out=logits[:], in0=pgate[:],
            in1=gbr_sb[:].unsqueeze(1).to_broadcast([P, NCH, E]),
            op=mybir.AluOpType.add)
        maxv = rpool.tile([P, NCH], F32, tag="maxv")
        nc.vector.tensor_reduce(maxv[:], logits[:], axis=mybir.AxisListType.X,
                                op=mybir.AluOpType.max)
        sh = rtmp.tile([P, NCH, E], F32, tag="tmp3d")
        nc.vector.tensor_tensor(out=sh[:], in0=logits[:],
                                in1=maxv[:].to_broadcast([P, NCH, E]),
                                op=mybir.AluOpType.subtract)
        ex = rtmp.tile([P, NCH, E], F32, tag="tmp3d")
        nc.scalar.activation(ex[:], sh[:], mybir.ActivationFunctionType.Exp)
        sumv = rpool.tile([P, NCH], F32, tag="sumv")
        nc.vector.tensor_reduce(sumv[:], ex[:], axis=mybir.AxisListType.X,
                                op=mybir.AluOpType.add)
        rec = rpool.tile([P, NCH], F32, tag="rec")
        nc.vector.reciprocal(rec[:], sumv[:])
        probs = rpool.tile([P, NCH, E], F32, tag="probs")
        nc.vector.tensor_tensor(out=probs[:], in0=ex[:],
                                in1=rec[:].to_broadcast([P, NCH, E]),
                                op=mybir.AluOpType.mult)
        # second max: mask out the argmax (same position for logits and probs)
        eq = rtmp.tile([P, NCH, E], F32, tag="tmp3d")
        nc.vector.tensor_tensor(out=eq[:], in0=logits[:],
                                in1=maxv[:].to_broadcast([P, NCH, E]),
                                op=mybir.AluOpType.is_ge)
        eqn = rtmp.tile([P, NCH, E], F32, tag="tmp3d")
        nc.vector.tensor_scalar(eqn[:], eq[:], -1.0, 1.0,
                                op0=mybir.AluOpType.mult, op1=mybir.AluOpType.add)
        pm = rtmp.tile([P, NCH, E], F32, tag="tmp3d")
        nc.vector.tensor_tensor(out=pm[:], in0=probs[:], in1=eqn[:],
                                op=mybir.AluOpType.mult)
        v2 = rpool.tile([P, NCH], F32, tag="v2")
        nc.vector.tensor_reduce(v2[:], pm[:], axis=mybir.AxisListType.X,
                                op=mybir.AluOpType.max)
        sel = rtmp.tile([P, NCH, E], F32, tag="tmp3d")
        nc.vector.tensor_tensor(out=sel[:], in0=probs[:],
                                in1=v2[:].to_broadcast([P, NCH, E]),
                                op=mybir.AluOpType.is_ge)
        wsel = rtmp.tile([P, NCH, E], F32, tag="tmp3d")
        nc.vector.tensor_tensor(out=wsel[:], in0=probs[:], in1=sel[:],
                                op=mybir.AluOpType.mult)
        wse = rtmp.tile([P, NCH, E], F32, tag="tmp3d")
        nc.vector.tensor_tensor(out=wse[:], in0=wsel[:], in1=esel_sb[:],
                                op=mybir.AluOpType.mult)
        wvec = rpool.tile([P, NCH], F32, tag="wvec")
        nc.vector.tensor_reduce(wvec[:], wse[:], axis=mybir.AxisListType.X,
                                op=mybir.AluOpType.add)
        msk = rpool.tile([P, NCH], F32, tag="msk")
        nc.vector.tensor_scalar(msk[:], wvec[:], 0.0, None,
                                op0=mybir.AluOpType.is_gt)

        # ---------------- phase 3: prefix-sum compaction ----------------
        incl_ps = pp.tile([P, NCH], F32, tag="bank")
        nc.tensor.matmul(incl_ps[:], lhsT=LT_sb[:], rhs=msk[:], start=True, stop=True)
        incl = rpool.tile([P, NCH], F32, tag="incl")
        nc.vector.tensor_copy(out=incl[:], in_=incl_ps[:])
        tot_ps = pp.tile([1, NCH], F32, tag="bank")
        nc.tensor.matmul(tot_ps[:], lhsT=LT_sb[:, P - 1:P], rhs=msk[:],
                         start=True, stop=True)
        tot = rpool.tile([1, NCH], F32, tag="tot")
        nc.vector.tensor_copy(out=tot[:], in_=tot_ps[:])
        ioff = rpool.tile([1, NCH], F32, tag="ioff")
        nc.vector.tensor_tensor_scan(ioff[:], tot[:], tot[:], 0.0,
                                     op0=mybir.AluOpType.add,
                                     op1=mybir.AluOpType.bypass)
        eoff = rpool.tile([1, NCH], F32, tag="eoff")
        nc.vector.tensor_tensor(out=eoff[:], in0=ioff[:], in1=tot[:],
                                op=mybir.AluOpType.subtract)
        coff_ps = pp.tile([P, NCH], F32, tag="bank")
        nc.tensor.matmul(coff_ps[:], lhsT=LT_sb[0:1, :], rhs=eoff[:],
                         start=True, stop=True)
        pos = rpool.tile([P, NCH], F32, tag="pos")
        nc.vector.tensor_tensor(out=pos[:], in0=incl[:], in1=msk[:],
                                op=mybir.AluOpType.subtract)
        nc.vector.tensor_tensor(out=pos[:], in0=pos[:], in1=coff_ps[:],
                                op=mybir.AluOpType.add)
        dst = rpool.tile([P, NCH], F32, tag="dst")
        nc.vector.tensor_scalar(dst[:], pos[:], float(CAP), None,
                                op0=mybir.AluOpType.subtract)
        nc.vector.tensor_tensor(out=dst[:], in0=dst[:], in1=msk[:],
                                op=mybir.AluOpType.mult)
        nc.vector.tensor_scalar(dst[:], dst[:], float(CAP), float(CAP),
                                op0=mybir.AluOpType.add, op1=mybir.AluOpType.min)
        dsti = rpool.tile([P, NCH], I32, tag="dsti")
        nc.vector.tensor_copy(out=dsti[:], in_=dst[:])
        iw = rpool.tile([P, NCH, 2], F32, tag="iw")
        nc.vector.tensor_copy(out=iw[:, :, 0], in_=tok_sb[:])
        nc.vector.tensor_copy(out=iw[:, :, 1], in_=wvec[:])

        # zero the packed table, then scatter
        z = rpool.tile([P, 2 * CAP // P], F32, tag="z")
        nc.vector.memset(z[:], 0.0)
        nc.sync.dma_start(
            out=IW[0:CAP, :].rearrange("(p a) b -> p (a b)", p=P), in_=z[:])
        nc.sync.dma_start(out=IW[CAP:CAP + 1, :], in_=z[0:1, 0:2])
        for j in range(NCH):
            nc.gpsimd.indirect_dma_start(
                out=IW[:],
                out_offset=IndirectOffsetOnAxis(ap=dsti[:, j:j + 1], axis=0),
                in_=iw[:, j, :],
                in_offset=None,
            )

        # ---------------- phase 4: gather + transpose ----------------
        xgT = xgTpool.tile([P, N // P, CAP], F16, tag="xgT")
        wpk = rpool.tile([P, CCH], F32, tag="wpk")
        for c in range(CCH):
            iwc = iwpool.tile([P, 2], F32, tag="iwc")
            nc.sync.dma_start(iwc[:], IW[c * P:(c + 1) * P, :])
            idx = iwpool.tile([P, 1], I32, tag="idx")
            nc.vector.tensor_copy(out=idx[:], in_=iwc[:, 0:1])
            nc.vector.tensor_copy(out=wpk[:, c:c + 1], in_=iwc[:, 1:2])
            xg = xgpool.tile([P, N], F16, tag="xg")
            nc.gpsimd.indirect_dma_start(
                out=xg[:], out_offset=None,
                in_=xr16[:],
                in_offset=IndirectOffsetOnAxis(ap=idx[:, 0:1], axis=0),
            )
            for e in range(N // P):
                tp = ptr.tile([P, P], F16, tag="tr")
                nc.tensor.transpose(tp[:], xg[:, e * P:(e + 1) * P], eye16_sb[:])
                nc.vector.tensor_copy(out=xgT[:, e, c * P:(c + 1) * P], in_=tp[:])

        # per-token gate weights broadcast across partitions: [P, CAP] f32.
        # DMA round-trip turns the per-partition wpk layout into a single-
        # partition free-dim row, then a K=1 ones-matmul broadcasts it.
        nc.sync.dma_start(out=wscr[:].rearrange("(c i) -> i c", i=P), in_=wpk[:])
        wfree = rpool.tile([1, CAP], F32, tag="wfree")
        nc.sync.dma_start(out=wfree[:], in_=wscr[:].rearrange("(a b) -> a b", a=1))
        wbc = wbcpool.tile([P, CAP], F32, tag="wbc")
        for (o, W) in TCH:
            bc_ps = pp.tile([P, 512], F32, tag="bank")
            nc.tensor.matmul(bc_ps[:, :W], lhsT=LT_sb[0:1, :], rhs=wfree[:, o:o + W],
                             start=True, stop=True)
            nc.vector.tensor_copy(out=wbc[:, o:o + W], in_=bc_ps[:, :W])

        # ---------------- phase 5: FFN layer 1 (h^T = relu(w1^T x^T + b1)) ----
        hT = hTpool.tile([P, H // P, CAP], F16, tag="hT")
        for hid in range(H // P):
            w1s = w1pool.tile([P, N // P, P], F16, tag="w1s")
            nc.sync.dma_start(w1s[:], w1r[hid])
            pss = []
            for (o, W) in TCH:
                ps = pp.tile([P, 512], F32, tag="bank")
                pss.append(ps)
            for a in range(N // P):
                for i, (o, W) in enumerate(TCH):
                    nc.tensor.matmul(pss[i][:, :W], lhsT=w1s[:, a, :],
                                     rhs=xgT[:, a, o:o + W],
                                     start=(a == 0), stop=(a == N // P - 1))
            for i, (o, W) in enumerate(TCH):
                nc.scalar.activation(hT[:, hid, o:o + W], pss[i][:, :W],
                                     mybir.ActivationFunctionType.Relu,
                                     bias=b1_sb[:, hid:hid + 1])

        # ---------------- phase 6: FFN layer 2 (y^T = (w2^T h^T + b2) * w) ----
        for e in range(N // P):
            w2s = w2pool.tile([P, H // P, P], F16, tag="w2s")
            nc.sync.dma_start(w2s[:], w2r[e])
            pss = []
            for (o, W) in TCH:
                ps = pp.tile([P, 512], F32, tag="bank")
                pss.append(ps)
            for a in range(H // P):
                for i, (o, W) in enumerate(TCH):
                    nc.tensor.matmul(pss[i][:, :W], lhsT=w2s[:, a, :],
                                     rhs=hT[:, a, o:o + W],
                                     start=(a == 0), stop=(a == H // P - 1))
            for i, (o, W) in enumerate(TCH):
                t = evpool.tile([P, 512], F32, tag="ev32")
                nc.vector.tensor_scalar(t[:, :W], pss[i][:, :W],
                                        b2_sb[:, e:e + 1], None,
                                        op0=mybir.AluOpType.add)
                y16 = evpool.tile([P, 512], F16, tag="ev16")
                nc.vector.tensor_tensor(out=y16[:, :W], in0=t[:, :W],
                                        in1=wbc[:, o:o + W],
                                        op=mybir.AluOpType.mult)
                nc.sync.dma_start(out=Y[e * P:(e + 1) * P, o:o + W],
                                  in_=y16[:, :W])

    nc.finalize()
    return nc


_NC_CACHE = None


def _get_nc():
    global _NC_CACHE
    if _NC_CACHE is None:
        _NC_CACHE = build_program()
    return _NC_CACHE


def make_in_maps(x, gate_w, gate_b, w1, b1, w2, b2):
    x2d = np.ascontiguousarray(np.asarray(x, np.float32).reshape(T, N))
    xT = np.ascontiguousarray(x2d.T)
    xr16 = x2d.astype(np.float16)
    gwr = np.ascontiguousarray(
        np.asarray(gate_w, np.float32).reshape(8, P, E).transpose(1, 0, 2))
    gbr = np.ascontiguousarray(
        np.broadcast_to(np.asarray(gate_b, np.float32)[None, :], (P, E)))
    LTm = (np.arange(P)[:, None] <= np.arange(P)[None, :]).astype(np.float32)
    TOK = (np.arange(NCH)[None, :] * P + np.arange(P)[:, None]).astype(np.float32)
    EYE16 = np.eye(P, dtype=np.float16)
    EYE32 = np.eye(P, dtype=np.float32)

    in_maps = []
    for p in range(E):
        w1p = np.asarray(w1[p], np.float32).astype(np.float16)  # [N, H]
        w2p = np.asarray(w2[p], np.float32).astype(np.float16)  # [H, N]
        w1r = np.ascontiguousarray(
            w1p.reshape(N // P, P, H // P, P).transpose(2, 1, 0, 3))
        w2r = np.ascontiguousarray(
            w2p.reshape(H // P, P, N // P, P).transpose(2, 1, 0, 3))
        b1p = np.asarray(b1[p], np.float32)
        b2p = np.asarray(b2[p], np.float32)
        b1r = np.ascontiguousarray(b1p.reshape(H // P, P).T)
        b2r = np.ascontiguousarray(b2p.reshape(N // P, P).T)
        onehot = np.zeros(E, np.float32)
        onehot[p] = 1.0
        esel = np.ascontiguousarray(
            np.broadcast_to(onehot[None, None, :], (P, NCH, E)).reshape(P, NCH * E))
        in_maps.append({
            "xT": xT, "xr16": xr16, "gwr": gwr, "gbr": gbr,
            "w1r": w1r, "w2r": w2r, "b1r": b1r, "b2r": b2r,
            "esel": esel, "LT": LTm, "TOKID": TOK,
            "EYE16": EYE16, "EYE32": EYE32,
        })
    return in_maps


def combine(results):
    """results: list of per-core dicts with Y [N, CAP] f16 and IW [CAP+1, 2] f32."""
    acc = np.zeros((T + 1, N), np.float32)
    for p in range(E):
        Yp = np.asarray(results[p]["Y"], np.float16).astype(np.float32).T  # [CAP, N]
        iw = np.asarray(results[p]["IW"], np.float32)[:CAP]
        idx = iw[:, 0].astype(np.int64)
        w = iw[:, 1]
        idx[w <= 0.0] = T  # pad slots -> trash row (their Y rows are exactly 0)
        assert idx.max() <= T and idx.min() >= 0
        acc[idx] += Yp
    return acc[:T].reshape(4, 2048, N)


def kernel(**inputs):
    nc = _get_nc()
    in_maps = make_in_maps(**inputs)
    res = run_bass_kernel_spmd(nc, in_maps, core_ids=list(range(E)))
    return combine(res.results).astype(np.float32)


if __name__ == "__main__":
    # smoke: build program only
    nc = build_program()
    print("program built OK")
